# revision 16
# baseline (speedup 1.0000x reference)
"""CantorAttention Trainium2 kernel (8 NeuronCores, SPMD), v2.

Strategy:
  - Shard batch (2) x head-groups (4 heads each) across the 8 cores.
  - Host: sort sequence positions by Cantor value so each 128-query block
    attends to a narrow contiguous band of keys; bands are expressed as
    absolute 128-row key tiles ("windows") so the banded V needs no
    repacking (PV matmuls read v_sb tiles directly).
  - Device per core, engineered for continuous PE occupancy (p-states):
      * interleaved per-kt DMA loads feeding a g-blocked q/k projection
        (8 PSUM accumulators, PE-weight reuse across 512-col matmuls),
      * v projection,
      * per-block attention: QK -> exp (head-pair fused) -> mask mul ->
        PV with a baked ones-column for the softmax denominator,
      * per-block normalize: DVE reciprocal of the denominator row +
        GpSimd partition_broadcast + DVE muls (no DRAM round trips),
      * per-block output projection draining straight into an fp16 tile
        that DMAs out, so outproj matmuls fill PE gaps during attention.
  - Host: sum the 4 per-batch partials, add b_out, un-permute rows.

Correct for arbitrary routes tables: windows/masks derive from the actual
routes input; the Cantor sort is only a (data-independent) heuristic that
keeps the windows tight for Cantor-routed inputs.
"""

import os
import sys

sys.path.insert(0, "/opt/trn_rl_repo")

import numpy as np
import ml_dtypes

import concourse.bass as bass
import concourse.mybir as mybir
import concourse.tile as tile
from concourse import bacc
from concourse.bass_utils import run_bass_kernel_spmd

B, S, DIM, H, HD, KNN, DEPTH = 2, 2048, 1024, 16, 64, 64, 8
SCALE = 1.0 / np.sqrt(HD)
N_CORES = 8
HPC = H // (N_CORES // B)       # heads per core = 4
FQK = 2 * HPC * HD              # q+k rows per core = 512
FV = HPC * HD                   # v rows per core = 256
BLK = 128                       # queries per attention block
NBLK = S // BLK                 # 16
KT = DIM // 128                 # 8 contraction tiles

F32 = mybir.dt.float32
F16 = mybir.dt.float16
BF16 = mybir.dt.bfloat16
BF16NP = ml_dtypes.bfloat16

LAST_RESULTS = None  # BassKernelResults of the most recent run (for test.py)
_PROGRAM_CACHE = {}


def _ensure_axon_hooks():
    """Provide antenv.axon_hooks if the image lacks it, wiring the NTFF
    profile hook from the boot shim so BASS_TRACE=1 can capture timings."""
    try:
        import antenv.axon_hooks  # noqa: F401
        return
    except ImportError:
        pass
    import types
    import antenv
    hook = None
    try:
        from trn_agent_boot.trn_boot import _ntff_profile_via_ctypes
        if os.path.exists("/opt/axon/libaxon_pjrt.so"):
            hook = _ntff_profile_via_ctypes("/opt/axon/libaxon_pjrt.so")
    except Exception:
        hook = None
    mod = types.ModuleType("antenv.axon_hooks")
    mod.get_axon_ntff_profile_hook = lambda: hook
    mod.set_axon_ntff_profile_hook = lambda h: None
    sys.modules["antenv.axon_hooks"] = mod
    antenv.axon_hooks = mod


def _patch_upload():
    """Don't attempt S3 artifact uploads from the sandbox."""
    import concourse.bass_utils as bu
    bu.upload_artifacts = lambda tmpdir: str(tmpdir)


_ensure_axon_hooks()
_patch_upload()


def _cantor_values(seq_len, depth):
    pos = np.arange(seq_len, dtype=np.float64)
    x = pos / max(1, seq_len - 1)
    x = np.clip(x, 1e-06, 1.0 - 1e-06)
    cantor = np.zeros(seq_len, dtype=np.float64)
    factor = 0.5
    for _ in range(depth):
        x = x * 3.0
        digit = np.floor(x)
        x = x - digit
        cantor += factor * (digit == 2.0)
        factor *= 0.5
    return cantor.astype(np.float32)


def _plan_windows(routes_p):
    """Per 128-query block: list of (key_tile, width) windows.

    Windows are absolute 128-row tiles of the key sequence; the first
    window of a block is always full-width, the last may be a 32-multiple
    prefix. Masks zero out any in-window key not routed, so stale PSUM
    rows in partial windows are harmless (exp of a bounded score * 0).
    """
    lo_all = routes_p.min(axis=1).reshape(NBLK, BLK).min(axis=1)
    hi_all = (routes_p.max(axis=1) + 1).reshape(NBLK, BLK).max(axis=1)
    plans = []
    for b in range(NBLK):
        lo = (int(lo_all[b]) // 128) * 128      # full first tile
        hi32 = int(np.ceil(hi_all[b] / 32.0)) * 32
        a0 = lo // 128
        aL = (hi32 - 1) // 128
        wins = []
        for t in range(a0, aL + 1):
            w = 128 if t < aL else hi32 - 128 * aL
            wins.append((t, w))
        plans.append(tuple(wins))
    return tuple(plans)


def _build_masks(routes_p, plans):
    """Count-masks in device layout [128, 2, nU, BLK] bf16 (head-pair dup,
    hh-major to match the bank-separated score layout).

    mask[p, :, u, q] = multiplicity of key (128*tile_u + p) in the routes
    of query q of the block owning window u.
    """
    parts = []
    for b, wins in enumerate(plans):
        r = routes_p[b * BLK:(b + 1) * BLK]              # [BLK, KNN]
        for (t, w) in wins:
            m = np.zeros((128, BLK), dtype=np.float32)
            rel = r - 128 * t
            sel = (rel >= 0) & (rel < 128)
            qidx = np.broadcast_to(np.arange(BLK)[:, None], rel.shape)
            np.add.at(m, (rel[sel], qidx[sel]), 1.0)
            parts.append(m)
    mk = np.stack(parts, axis=1)                         # [128, nU, BLK]
    mk = np.broadcast_to(mk[:, None, :, :], (128, 2, mk.shape[1], BLK))
    return np.ascontiguousarray(mk).astype(BF16NP)


def _build_program(plans):
    """Emit the SPMD Bass program for the given window plan."""
    nU = sum(len(w) for w in plans)
    nwmax = max(len(w) for w in plans)
    # scores psum tile is [128, 2, 4, BLK]: each hh (PE row group) owns its
    # own PSUM bank — concurrent row-group matmuls must not share a bank.
    assert nwmax <= 4, f"window plan too wide for psum banking: {nwmax}"

    nc = bacc.Bacc("TRN2", target_bir_lowering=False)

    xT_d = nc.dram_tensor("xT", [DIM, S], BF16, kind="ExternalInput")
    wq_d = nc.dram_tensor("wqkvT", [DIM, FQK + FV], BF16, kind="ExternalInput")
    bqk_d = nc.dram_tensor("bqk", [FQK], F32, kind="ExternalInput")
    bv_d = nc.dram_tensor("bv2", [2 * FV], F32, kind="ExternalInput")
    wo_d = nc.dram_tensor("woT", [FV, DIM], BF16, kind="ExternalInput")
    mask_d = nc.dram_tensor("maskT", [128, 2, nU, BLK], BF16, kind="ExternalInput")
    out_d = nc.dram_tensor("out_p", [S, DIM], F16, kind="ExternalOutput")

    with tile.TileContext(nc) as tc:
        with tc.tile_pool(name="const", bufs=1) as cpool, \
             tc.tile_pool(name="work", bufs=1) as wpool, \
             tc.tile_pool(name="epool", bufs=3) as epool:

            # ---- constant loads, interleaved per contraction tile ----
            xT = cpool.tile([128, KT, S], BF16, tag="xT")
            wq = cpool.tile([128, KT, FQK + FV], BF16, tag="wq")
            for kt in range(KT):
                nc.sync.dma_start(
                    wq[:, kt, :],
                    wq_d.rearrange("(t p) f -> p t f", p=128)[:, kt, :])
                nc.sync.dma_start(
                    xT[:, kt, :],
                    xT_d.rearrange("(t p) s -> p t s", p=128)[:, kt, :])
            bqk = cpool.tile([128, FQK // 128], F32, tag="bqk")
            nc.sync.dma_start(bqk[:], bqk_d.rearrange("(t p) -> p t", p=128))
            # v bias pre-duplicated for fused adds over st pairs
            bvb = cpool.tile([128, 2, HPC, HD], F32, tag="bvb")
            nc.sync.dma_start(
                bvb[:],
                bv_d[None, :].to_broadcast((128, 2 * FV)).rearrange(
                    "p (j h d) -> p j h d", j=2, h=HPC))
            wo = cpool.tile([128, 2, DIM], BF16, tag="wo")
            nc.sync.dma_start(wo[:], wo_d.rearrange("(t p) o -> p t o", p=128))
            mk = wpool.tile([128, 2, nU, BLK], BF16, tag="mask")
            nc.sync.dma_start(mk[:], mask_d[:])

            qk_sb = wpool.tile([128, FQK // 128, S], BF16, tag="qk")
            # v layout per head: col 0 = ones (softmax denominator lands on
            # PSUM partition 0), cols 1..63 = zero pad (so attention values
            # land at partition 64 — engine APs spanning 64 partitions must
            # start at partition 0 or 64), cols 64..127 = v.
            VW = 128
            v_sb = wpool.tile([128, S // 128, HPC, VW], BF16, tag="v")
            attnT = wpool.tile([128, 2, S], BF16, tag="attnT")
            nc.gpsimd.memset(v_sb[:, :, :, 0:1], 1.0)
            nc.gpsimd.memset(v_sb[:, :, :, 1:64], 0.0)

            # v tiles required before each block's PV can run
            need_tile = [max(t for t, _ in plans[b]) for b in range(NBLK)]
            for b in range(1, NBLK):
                need_tile[b] = max(need_tile[b], need_tile[b - 1])

            # Everything below runs in one PSUM regime: scores (4 banks),
            # attention-out (2), and a shared ring (2) for the projection
            # units and output projection. The q/k/v projections are
            # decomposed into 512-col units emitted between attention
            # blocks so the PE stays dense (HAM keeps it at full clock).
            with tc.tile_pool(name="ps", bufs=2, space="PSUM") as ps, \
                 tc.tile_pool(name="pv", bufs=2, space="PSUM") as pv, \
                 tc.tile_pool(name="po", bufs=2, space="PSUM") as po:

                def aunit(ft, st):
                    """q/k projection unit: qk_sb[:, ft, st*512:...]"""
                    pt = po.tile([128, 512], F32, tag="po",
                                 name=f"pa{ft}{st}")
                    for kt in range(KT):
                        nc.tensor.matmul(
                            pt[:],
                            wq[:, kt, ft * 128:(ft + 1) * 128],
                            xT[:, kt, st * 512:(st + 1) * 512],
                            start=(kt == 0), stop=(kt == KT - 1))
                    nc.scalar.activation(
                        qk_sb[:, ft, st * 512:(st + 1) * 512], pt[:],
                        mybir.ActivationFunctionType.Identity,
                        bias=bqk[:, ft:ft + 1])

                def vproj(sp):
                    """v projection for seq tiles 2sp, 2sp+1"""
                    pt = po.tile([128, 2, 256], F32, tag="po",
                                 name=f"pb{sp}")
                    for j in range(2):
                        st = 2 * sp + j
                        for kt in range(KT):
                            nc.tensor.matmul(
                                pt[:, j, :],
                                xT[:, kt, st * 128:(st + 1) * 128],
                                wq[:, kt, FQK:],
                                start=(kt == 0), stop=(kt == KT - 1))
                    nc.vector.tensor_add(
                        v_sb[:, 2 * sp:2 * sp + 2, :, 64:64 + HD],
                        pt.rearrange("p j (h d) -> p j h d", h=HPC),
                        bvb[:])

                done_q = 0   # q units emitted (st granularity, both ft)
                done_k = 0   # k units emitted
                done_v = 0   # v pairs emitted

                def ensure(q_st, k_tile, v_tile):
                    nonlocal done_q, done_k, done_v
                    while done_q <= min(q_st, S // 512 - 1):
                        aunit(0, done_q)
                        aunit(1, done_q)
                        done_q += 1
                    while done_k * 4 <= min(k_tile, S // 128 - 1):
                        aunit(2, done_k)
                        aunit(3, done_k)
                        done_k += 1
                    while done_v * 2 <= min(v_tile, S // 128 - 1):
                        vproj(done_v)
                        done_v += 1

                # prologue: what block 0 needs
                ensure(0, need_tile[0], need_tile[0])

                moff = 0
                for b in range(NBLK):
                    # stay one block group ahead of the attention consumers
                    nb = min(b + 1, NBLK - 1)
                    ensure(nb // 4, need_tile[nb], need_tile[nb])
                    if b == NBLK - 1:
                        ensure(S // 512 - 1, S // 128 - 1, S // 128 - 1)
                    wins = plans[b]
                    nw = len(wins)
                    qs = slice(b * BLK, (b + 1) * BLK)
                    pvt = pv.tile([128, 4, BLK], F32, tag="pv",
                                  name=f"pv{b}")
                    for hp in range(2):
                        pst = ps.tile([128, 2, 4, BLK], F32, tag="ps",
                                      name=f"ps{b}{hp}")
                        for hh in range(2):
                            for u, (t, w) in enumerate(wins):
                                nc.tensor.matmul(
                                    pst[0:w, hh, u, :],
                                    qk_sb[64 * hh:64 * hh + 64, 2 + hp,
                                          128 * t:128 * t + w],
                                    qk_sb[64 * hh:64 * hh + 64, hp, qs],
                                    start=True, stop=True)
                        et = epool.tile([128, 2, 4, BLK], BF16, tag="et")
                        nc.scalar.activation(
                            et[:, :, :nw, :], pst[:, :, :nw, :],
                            mybir.ActivationFunctionType.Exp,
                            scale=float(SCALE))
                        emt = epool.tile([128, 2, 4, BLK], BF16, tag="emt")
                        eng = nc.gpsimd if hp == 0 else nc.vector
                        eng.tensor_mul(emt[:, :, :nw, :],
                                       et[:, :, :nw, :],
                                       mk[:, :, moff:moff + nw, :])
                        for hh in range(2):
                            for u, (t, w) in enumerate(wins):
                                nc.tensor.matmul(
                                    pvt[0:VW, 2 * hp + hh, :],
                                    v_sb[0:w, t, 2 * hp + hh, :],
                                    emt[0:w, hh, u, :],
                                    start=(u == 0), stop=(u == nw - 1))
                    # normalize once per block: den rows are psum partition
                    # 0 of all four (hp, hh) slots
                    rr = epool.tile([1, 4, BLK], F32, tag="rr")
                    nc.vector.reciprocal_approx_fast(rr[:], pvt[0:1, :, :])
                    rb = epool.tile([128, 4, BLK], F32, tag="rb")
                    nc.gpsimd.partition_broadcast(rb[:], rr[:])
                    for hh in range(2):
                        nc.vector.tensor_mul(
                            attnT[64 * hh:64 * hh + 64, :, qs],
                            pvt[64:64 + HD, hh::2, :],
                            rb[64 * hh:64 * hh + 64, hh::2, :])
                    # output projection for this block's 128 queries
                    ob = epool.tile([128, DIM], F16, tag="ob")
                    for ot in range(2):
                        pot = po.tile([128, 512], F32, tag="po",
                                      name=f"po{b}{ot}")
                        for dt in range(2):
                            nc.tensor.matmul(
                                pot[:],
                                attnT[:, dt, qs],
                                wo[:, dt, ot * 512:(ot + 1) * 512],
                                start=(dt == 0), stop=(dt == 1))
                        if ot == 0:
                            nc.vector.tensor_copy(ob[:, :512], pot[:])
                        else:
                            nc.scalar.copy(ob[:, 512:], pot[:])
                    nc.sync.dma_start(out_d[qs, :], ob[:])
                    moff += nw

    nc.finalize()
    return nc


def kernel(x, w_qkv, b_qkv, w_out, b_out, routes):
    global LAST_RESULTS
    x = np.asarray(x, dtype=np.float32)
    w_qkv = np.asarray(w_qkv, dtype=np.float32)
    b_qkv = np.asarray(b_qkv, dtype=np.float32)
    w_out = np.asarray(w_out, dtype=np.float32)
    b_out = np.asarray(b_out, dtype=np.float32)
    routes = np.asarray(routes)

    # --- host: permutation + windows + masks ---
    cantor = _cantor_values(S, DEPTH)
    perm = np.lexsort((np.arange(S), cantor))
    inv_perm = np.empty(S, dtype=np.int64)
    inv_perm[perm] = np.arange(S)
    routes_p = inv_perm[routes.astype(np.int64)[perm]]
    plans = _plan_windows(routes_p)
    maskT = _build_masks(routes_p, plans)

    key = plans
    if key not in _PROGRAM_CACHE:
        _PROGRAM_CACHE[key] = _build_program(plans)
    nc = _PROGRAM_CACHE[key]

    # --- host: per-core inputs ---
    x_p = x[:, perm, :]                                   # [B, S, DIM]
    in_maps = []
    for c in range(N_CORES):
        b = c // (N_CORES // B)
        hg = c % (N_CORES // B)
        heads = range(hg * HPC, (hg + 1) * HPC)
        # w rows: q heads, k heads, v heads
        rows = ([h * HD + i for h in heads for i in range(HD)]
                + [DIM + h * HD + i for h in heads for i in range(HD)]
                + [2 * DIM + h * HD + i for h in heads for i in range(HD)])
        rows = np.asarray(rows)
        wq_c = np.ascontiguousarray(w_qkv[rows].T).astype(BF16NP)   # [1024, 768]
        bqk_c = np.ascontiguousarray(b_qkv[rows[:FQK]]).astype(np.float32)
        bv_c = np.ascontiguousarray(b_qkv[rows[FQK:]]).astype(np.float32)
        bv2_c = np.concatenate([bv_c, bv_c])
        wo_c = np.ascontiguousarray(
            w_out[:, hg * FV:(hg + 1) * FV].T).astype(BF16NP)
        in_maps.append({
            "xT": np.ascontiguousarray(x_p[b].T).astype(BF16NP),
            "wqkvT": wq_c,
            "bqk": bqk_c,
            "bv2": bv2_c,
            "woT": wo_c,
            "maskT": maskT,
        })

    try:
        res = run_bass_kernel_spmd(nc, in_maps, core_ids=list(range(N_CORES)))
    except Exception:
        if os.environ.get("BASS_TRACE"):
            # tracing infra failure — retry without profiling
            os.environ["BASS_NEVER_TRACE"] = "1"
            res = run_bass_kernel_spmd(nc, in_maps, core_ids=list(range(N_CORES)))
        else:
            raise
    LAST_RESULTS = res

    out = np.zeros((B, S, DIM), dtype=np.float32)
    for c in range(N_CORES):
        out[c // (N_CORES // B)] += res.results[c]["out_p"].astype(np.float32)
    out += b_out[None, None, :]
    out = out[:, inv_perm, :]    # un-permute rows
    return out


# revision 17
# speedup vs baseline: 1.9047x; 1.9047x over previous
"""CantorAttention Trainium2 kernel (8 NeuronCores, SPMD), v2.

Strategy:
  - Shard batch (2) x head-groups (4 heads each) across the 8 cores.
  - Host: sort sequence positions by Cantor value so each 128-query block
    attends to a narrow contiguous band of keys; bands are expressed as
    absolute 128-row key tiles ("windows") so the banded V needs no
    repacking (PV matmuls read v_sb tiles directly).
  - Device per core, engineered for continuous PE occupancy (p-states):
      * interleaved per-kt DMA loads feeding a g-blocked q/k projection
        (8 PSUM accumulators, PE-weight reuse across 512-col matmuls),
      * v projection,
      * per-block attention: QK -> exp (head-pair fused) -> mask mul ->
        PV with a baked ones-column for the softmax denominator,
      * per-block normalize: DVE reciprocal of the denominator row +
        GpSimd partition_broadcast + DVE muls (no DRAM round trips),
      * per-block output projection draining straight into an fp16 tile
        that DMAs out, so outproj matmuls fill PE gaps during attention.
  - Host: sum the 4 per-batch partials, add b_out, un-permute rows.

Correct for arbitrary routes tables: windows/masks derive from the actual
routes input; the Cantor sort is only a (data-independent) heuristic that
keeps the windows tight for Cantor-routed inputs.
"""

import os
import sys

sys.path.insert(0, "/opt/trn_rl_repo")

import numpy as np
import ml_dtypes

import concourse.bass as bass
import concourse.mybir as mybir
import concourse.tile as tile
from concourse import bacc
from concourse.bass_utils import run_bass_kernel_spmd

B, S, DIM, H, HD, KNN, DEPTH = 2, 2048, 1024, 16, 64, 64, 8
SCALE = 1.0 / np.sqrt(HD)
N_CORES = 8
HPC = H // (N_CORES // B)       # heads per core = 4
FQK = 2 * HPC * HD              # q+k rows per core = 512
FV = HPC * HD                   # v rows per core = 256
BLK = 128                       # queries per attention block
NBLK = S // BLK                 # 16
KT = DIM // 128                 # 8 contraction tiles

F32 = mybir.dt.float32
F16 = mybir.dt.float16
BF16 = mybir.dt.bfloat16
BF16NP = ml_dtypes.bfloat16

LAST_RESULTS = None  # BassKernelResults of the most recent run (for test.py)
_PROGRAM_CACHE = {}


def _ensure_axon_hooks():
    """Provide antenv.axon_hooks if the image lacks it, wiring the NTFF
    profile hook from the boot shim so BASS_TRACE=1 can capture timings."""
    try:
        import antenv.axon_hooks  # noqa: F401
        return
    except ImportError:
        pass
    import types
    import antenv
    hook = None
    try:
        from trn_agent_boot.trn_boot import _ntff_profile_via_ctypes
        if os.path.exists("/opt/axon/libaxon_pjrt.so"):
            hook = _ntff_profile_via_ctypes("/opt/axon/libaxon_pjrt.so")
    except Exception:
        hook = None
    mod = types.ModuleType("antenv.axon_hooks")
    mod.get_axon_ntff_profile_hook = lambda: hook
    mod.set_axon_ntff_profile_hook = lambda h: None
    sys.modules["antenv.axon_hooks"] = mod
    antenv.axon_hooks = mod


def _patch_upload():
    """Don't attempt S3 artifact uploads from the sandbox."""
    import concourse.bass_utils as bu
    bu.upload_artifacts = lambda tmpdir: str(tmpdir)


_ensure_axon_hooks()
_patch_upload()


def _cantor_values(seq_len, depth):
    pos = np.arange(seq_len, dtype=np.float64)
    x = pos / max(1, seq_len - 1)
    x = np.clip(x, 1e-06, 1.0 - 1e-06)
    cantor = np.zeros(seq_len, dtype=np.float64)
    factor = 0.5
    for _ in range(depth):
        x = x * 3.0
        digit = np.floor(x)
        x = x - digit
        cantor += factor * (digit == 2.0)
        factor *= 0.5
    return cantor.astype(np.float32)


def _plan_windows(routes_p):
    """Per 128-query block: list of (key_tile, width) windows.

    Windows are absolute 128-row tiles of the key sequence; the first
    window of a block is always full-width, the last may be a 32-multiple
    prefix. Masks zero out any in-window key not routed, so stale PSUM
    rows in partial windows are harmless (exp of a bounded score * 0).
    """
    lo_all = routes_p.min(axis=1).reshape(NBLK, BLK).min(axis=1)
    hi_all = (routes_p.max(axis=1) + 1).reshape(NBLK, BLK).max(axis=1)
    plans = []
    for b in range(NBLK):
        lo = (int(lo_all[b]) // 128) * 128      # full first tile
        hi32 = int(np.ceil(hi_all[b] / 32.0)) * 32
        a0 = lo // 128
        aL = (hi32 - 1) // 128
        wins = []
        for t in range(a0, aL + 1):
            w = 128 if t < aL else hi32 - 128 * aL
            wins.append((t, w))
        plans.append(tuple(wins))
    return tuple(plans)


def _build_masks(routes_p, plans):
    """Count-masks in device layout [128, 2, nU, BLK] bf16 (head-pair dup,
    hh-major to match the bank-separated score layout).

    mask[p, :, u, q] = multiplicity of key (128*tile_u + p) in the routes
    of query q of the block owning window u.
    """
    parts = []
    for b, wins in enumerate(plans):
        r = routes_p[b * BLK:(b + 1) * BLK]              # [BLK, KNN]
        for (t, w) in wins:
            m = np.zeros((128, BLK), dtype=np.float32)
            rel = r - 128 * t
            sel = (rel >= 0) & (rel < 128)
            qidx = np.broadcast_to(np.arange(BLK)[:, None], rel.shape)
            np.add.at(m, (rel[sel], qidx[sel]), 1.0)
            parts.append(m)
    mk = np.stack(parts, axis=1)                         # [128, nU, BLK]
    mk = np.broadcast_to(mk[:, None, :, :], (128, 2, mk.shape[1], BLK))
    return np.ascontiguousarray(mk).astype(BF16NP)


def _build_program(plans):
    """Emit the SPMD Bass program for the given window plan."""
    nU = sum(len(w) for w in plans)
    nwmax = max(len(w) for w in plans)
    # scores psum tile is [128, 2, 4, BLK]: each hh (PE row group) owns its
    # own PSUM bank — concurrent row-group matmuls must not share a bank.
    assert nwmax <= 4, f"window plan too wide for psum banking: {nwmax}"

    nc = bacc.Bacc("TRN2", target_bir_lowering=False)

    xT_d = nc.dram_tensor("xT", [DIM, S], BF16, kind="ExternalInput")
    wq_d = nc.dram_tensor("wqkvT", [DIM, FQK + FV], BF16, kind="ExternalInput")
    bqk_d = nc.dram_tensor("bqk", [FQK], F32, kind="ExternalInput")
    bv_d = nc.dram_tensor("bv2", [2 * FV], F32, kind="ExternalInput")
    wo_d = nc.dram_tensor("woT", [FV, DIM], BF16, kind="ExternalInput")
    mask_d = nc.dram_tensor("maskT", [128, 2, nU, BLK], BF16, kind="ExternalInput")
    out_d = nc.dram_tensor("out_p", [S, DIM], F16, kind="ExternalOutput")

    with tile.TileContext(nc) as tc:
        with tc.tile_pool(name="const", bufs=1) as cpool, \
             tc.tile_pool(name="work", bufs=1) as wpool, \
             tc.tile_pool(name="epool", bufs=3) as epool:

            # ---- constant loads, interleaved per contraction tile ----
            xT = cpool.tile([128, KT, S], BF16, tag="xT")
            wq = cpool.tile([128, KT, FQK + FV], BF16, tag="wq")
            for kt in range(KT):
                nc.sync.dma_start(
                    wq[:, kt, :],
                    wq_d.rearrange("(t p) f -> p t f", p=128)[:, kt, :])
                nc.sync.dma_start(
                    xT[:, kt, :],
                    xT_d.rearrange("(t p) s -> p t s", p=128)[:, kt, :])
            bqk = cpool.tile([128, FQK // 128], F32, tag="bqk")
            nc.sync.dma_start(bqk[:], bqk_d.rearrange("(t p) -> p t", p=128))
            # v bias pre-duplicated for fused adds over st pairs
            bvb = cpool.tile([128, 2, HPC, HD], F32, tag="bvb")
            nc.sync.dma_start(
                bvb[:],
                bv_d[None, :].to_broadcast((128, 2 * FV)).rearrange(
                    "p (j h d) -> p j h d", j=2, h=HPC))
            wo = cpool.tile([128, 2, DIM], BF16, tag="wo")
            nc.sync.dma_start(wo[:], wo_d.rearrange("(t p) o -> p t o", p=128))
            mk = wpool.tile([128, 2, nU, BLK], BF16, tag="mask")
            nc.sync.dma_start(mk[:], mask_d[:])

            qk_sb = wpool.tile([128, FQK // 128, S], BF16, tag="qk")
            # v layout per head: col 0 = ones (softmax denominator lands on
            # PSUM partition 0), cols 1..63 = zero pad (so attention values
            # land at partition 64 — engine APs spanning 64 partitions must
            # start at partition 0 or 64), cols 64..127 = v.
            VW = 128
            v_sb = wpool.tile([128, S // 128, HPC, VW], BF16, tag="v")
            attnT = wpool.tile([128, 2, S], BF16, tag="attnT")
            nc.gpsimd.memset(v_sb[:, :, :, 0:1], 1.0)
            nc.gpsimd.memset(v_sb[:, :, :, 1:64], 0.0)

            # v tiles required before each block's PV can run
            need_tile = [max(t for t, _ in plans[b]) for b in range(NBLK)]
            for b in range(1, NBLK):
                need_tile[b] = max(need_tile[b], need_tile[b - 1])
            next_sp = 0

            # v projection emitted as dense PE filler: prologue tiles in
            # the phase-A pool, the rest between attention blocks so the
            # HAM activity monitor keeps the PE at full clock.
            def vproj(sp, pool, tag):
                pt = pool.tile([128, 2, 256], F32, tag=tag, name=f"pb{sp}")
                for j in range(2):
                    st = 2 * sp + j
                    for kt in range(KT):
                        nc.tensor.matmul(
                            pt[:, j, :],
                            xT[:, kt, st * 128:(st + 1) * 128],
                            wq[:, kt, FQK:],
                            start=(kt == 0), stop=(kt == KT - 1))
                nc.vector.tensor_add(
                    v_sb[:, 2 * sp:2 * sp + 2, :, 64:64 + HD],
                    pt.rearrange("p j (h d) -> p j h d", h=HPC),
                    bvb[:])

            # ---- phase A: q/k projection -> qk_sb [128, 4, S] ----
            # g-blocked: 8 live psum accumulators, PE weights reused over
            # the 4 512-col column tiles, DMA-paced by the per-kt loads.
            with tc.tile_pool(name="pacc", bufs=8, space="PSUM") as pa:
                for g in range(2):
                    pts = []
                    for f in range(2):
                        for st in range(4):
                            t = pa.tile([128, 512], F32, tag="acc",
                                        name=f"pa{g}{f}{st}")
                            pts.append(t)
                    for kt in range(KT):
                        for f in range(2):
                            ft = 2 * g + f
                            for st in range(4):
                                nc.tensor.matmul(
                                    pts[f * 4 + st][:],
                                    wq[:, kt, ft * 128:(ft + 1) * 128],
                                    xT[:, kt, st * 512:(st + 1) * 512],
                                    start=(kt == 0), stop=(kt == KT - 1))
                    for f in range(2):
                        ft = 2 * g + f
                        for st in range(4):
                            nc.scalar.activation(
                                qk_sb[:, ft, st * 512:(st + 1) * 512],
                                pts[f * 4 + st][:],
                                mybir.ActivationFunctionType.Identity,
                                bias=bqk[:, ft:ft + 1])
                # v-projection prologue: tiles block 0 needs, emitted in
                # this pool so the PE stays dense across the phase boundary
                while next_sp * 2 <= need_tile[0] and next_sp < S // 256:
                    vproj(next_sp, pa, "acc")
                    next_sp += 1

            # ---- attention + v-projection + outproj, per block ----
            with tc.tile_pool(name="ps", bufs=2, space="PSUM") as ps, \
                 tc.tile_pool(name="pv", bufs=2, space="PSUM") as pv, \
                 tc.tile_pool(name="po", bufs=2, space="PSUM") as po:
                moff = 0
                for b in range(NBLK):
                    # keep one block of v-projection lookahead
                    need = need_tile[min(b + 1, NBLK - 1)]
                    if b == NBLK - 1:
                        need = S // 128 - 1
                    while next_sp * 2 <= need and next_sp < S // 256:
                        vproj(next_sp, po, "po")
                        next_sp += 1
                    wins = plans[b]
                    nw = len(wins)
                    qs = slice(b * BLK, (b + 1) * BLK)
                    pvt = pv.tile([128, 4, BLK], F32, tag="pv",
                                  name=f"pv{b}")
                    for hp in range(2):
                        pst = ps.tile([128, 2, 4, BLK], F32, tag="ps",
                                      name=f"ps{b}{hp}")
                        for hh in range(2):
                            for u, (t, w) in enumerate(wins):
                                nc.tensor.matmul(
                                    pst[0:w, hh, u, :],
                                    qk_sb[64 * hh:64 * hh + 64, 2 + hp,
                                          128 * t:128 * t + w],
                                    qk_sb[64 * hh:64 * hh + 64, hp, qs],
                                    start=True, stop=True)
                        et = epool.tile([128, 2, 4, BLK], BF16, tag="et")
                        nc.scalar.activation(
                            et[:, :, :nw, :], pst[:, :, :nw, :],
                            mybir.ActivationFunctionType.Exp,
                            scale=float(SCALE))
                        emt = epool.tile([128, 2, 4, BLK], BF16, tag="emt")
                        nc.vector.tensor_mul(emt[:, :, :nw, :],
                                             et[:, :, :nw, :],
                                             mk[:, :, moff:moff + nw, :])
                        for hh in range(2):
                            for u, (t, w) in enumerate(wins):
                                nc.tensor.matmul(
                                    pvt[0:VW, 2 * hp + hh, :],
                                    v_sb[0:w, t, 2 * hp + hh, :],
                                    emt[0:w, hh, u, :],
                                    start=(u == 0), stop=(u == nw - 1))
                    # normalize once per block: den rows are psum partition
                    # 0 of all four (hp, hh) slots
                    rr = epool.tile([1, 4, BLK], F32, tag="rr")
                    nc.vector.reciprocal_approx_fast(rr[:], pvt[0:1, :, :])
                    rb = epool.tile([128, 4, BLK], F32, tag="rb")
                    nc.gpsimd.partition_broadcast(rb[:], rr[:])
                    for hh in range(2):
                        nc.vector.tensor_mul(
                            attnT[64 * hh:64 * hh + 64, :, qs],
                            pvt[64:64 + HD, hh::2, :],
                            rb[64 * hh:64 * hh + 64, hh::2, :])
                    # output projection for this block's 128 queries
                    ob = epool.tile([128, DIM], F16, tag="ob")
                    for ot in range(2):
                        pot = po.tile([128, 512], F32, tag="po",
                                      name=f"po{b}{ot}")
                        for dt in range(2):
                            nc.tensor.matmul(
                                pot[:],
                                attnT[:, dt, qs],
                                wo[:, dt, ot * 512:(ot + 1) * 512],
                                start=(dt == 0), stop=(dt == 1))
                        nc.scalar.copy(ob[:, ot * 512:(ot + 1) * 512],
                                       pot[:])
                    nc.sync.dma_start(out_d[qs, :], ob[:])
                    moff += nw

    nc.finalize()
    return nc


def kernel(x, w_qkv, b_qkv, w_out, b_out, routes):
    global LAST_RESULTS
    x = np.asarray(x, dtype=np.float32)
    w_qkv = np.asarray(w_qkv, dtype=np.float32)
    b_qkv = np.asarray(b_qkv, dtype=np.float32)
    w_out = np.asarray(w_out, dtype=np.float32)
    b_out = np.asarray(b_out, dtype=np.float32)
    routes = np.asarray(routes)

    # --- host: permutation + windows + masks ---
    cantor = _cantor_values(S, DEPTH)
    perm = np.lexsort((np.arange(S), cantor))
    inv_perm = np.empty(S, dtype=np.int64)
    inv_perm[perm] = np.arange(S)
    routes_p = inv_perm[routes.astype(np.int64)[perm]]
    plans = _plan_windows(routes_p)
    maskT = _build_masks(routes_p, plans)

    key = plans
    if key not in _PROGRAM_CACHE:
        _PROGRAM_CACHE[key] = _build_program(plans)
    nc = _PROGRAM_CACHE[key]

    # --- host: per-core inputs ---
    x_p = x[:, perm, :]                                   # [B, S, DIM]
    in_maps = []
    for c in range(N_CORES):
        b = c // (N_CORES // B)
        hg = c % (N_CORES // B)
        heads = range(hg * HPC, (hg + 1) * HPC)
        # w rows: q heads, k heads, v heads
        rows = ([h * HD + i for h in heads for i in range(HD)]
                + [DIM + h * HD + i for h in heads for i in range(HD)]
                + [2 * DIM + h * HD + i for h in heads for i in range(HD)])
        rows = np.asarray(rows)
        wq_c = np.ascontiguousarray(w_qkv[rows].T).astype(BF16NP)   # [1024, 768]
        bqk_c = np.ascontiguousarray(b_qkv[rows[:FQK]]).astype(np.float32)
        bv_c = np.ascontiguousarray(b_qkv[rows[FQK:]]).astype(np.float32)
        bv2_c = np.concatenate([bv_c, bv_c])
        wo_c = np.ascontiguousarray(
            w_out[:, hg * FV:(hg + 1) * FV].T).astype(BF16NP)
        in_maps.append({
            "xT": np.ascontiguousarray(x_p[b].T).astype(BF16NP),
            "wqkvT": wq_c,
            "bqk": bqk_c,
            "bv2": bv2_c,
            "woT": wo_c,
            "maskT": maskT,
        })

    try:
        res = run_bass_kernel_spmd(nc, in_maps, core_ids=list(range(N_CORES)))
    except Exception:
        if os.environ.get("BASS_TRACE"):
            # tracing infra failure — retry without profiling
            os.environ["BASS_NEVER_TRACE"] = "1"
            res = run_bass_kernel_spmd(nc, in_maps, core_ids=list(range(N_CORES)))
        else:
            raise
    LAST_RESULTS = res

    out = np.zeros((B, S, DIM), dtype=np.float32)
    for c in range(N_CORES):
        out[c // (N_CORES // B)] += res.results[c]["out_p"].astype(np.float32)
    out += b_out[None, None, :]
    out = out[:, inv_perm, :]    # un-permute rows
    return out


# revision 19
# speedup vs baseline: 2.1991x; 1.1546x over previous
"""CantorAttention Trainium2 kernel (8 NeuronCores, SPMD), v2.

Strategy:
  - Shard batch (2) x head-groups (4 heads each) across the 8 cores.
  - Host: sort sequence positions by Cantor value so each 128-query block
    attends to a narrow contiguous band of keys; bands are expressed as
    absolute 128-row key tiles ("windows") so the banded V needs no
    repacking (PV matmuls read v_sb tiles directly).
  - Device per core, engineered for continuous PE occupancy (p-states):
      * interleaved per-kt DMA loads feeding a g-blocked q/k projection
        (8 PSUM accumulators, PE-weight reuse across 512-col matmuls),
      * v projection,
      * per-block attention: QK -> exp (head-pair fused) -> mask mul ->
        PV with a baked ones-column for the softmax denominator,
      * per-block normalize: DVE reciprocal of the denominator row +
        GpSimd partition_broadcast + DVE muls (no DRAM round trips),
      * per-block output projection draining straight into an fp16 tile
        that DMAs out, so outproj matmuls fill PE gaps during attention.
  - Host: sum the 4 per-batch partials, add b_out, un-permute rows.

Correct for arbitrary routes tables: windows/masks derive from the actual
routes input; the Cantor sort is only a (data-independent) heuristic that
keeps the windows tight for Cantor-routed inputs.
"""

import os
import sys

sys.path.insert(0, "/opt/trn_rl_repo")

import numpy as np
import ml_dtypes

import concourse.bass as bass
import concourse.mybir as mybir
import concourse.tile as tile
from concourse import bacc
from concourse.bass_utils import run_bass_kernel_spmd

B, S, DIM, H, HD, KNN, DEPTH = 2, 2048, 1024, 16, 64, 64, 8
SCALE = 1.0 / np.sqrt(HD)
N_CORES = 8
HPC = H // (N_CORES // B)       # heads per core = 4
FQK = 2 * HPC * HD              # q+k rows per core = 512
FV = HPC * HD                   # v rows per core = 256
BLK = 128                       # queries per attention block
NBLK = S // BLK                 # 16
KT = DIM // 128                 # 8 contraction tiles

F32 = mybir.dt.float32
F16 = mybir.dt.float16
BF16 = mybir.dt.bfloat16
BF16NP = ml_dtypes.bfloat16

LAST_RESULTS = None  # BassKernelResults of the most recent run (for test.py)
_PROGRAM_CACHE = {}


def _ensure_axon_hooks():
    """Provide antenv.axon_hooks if the image lacks it, wiring the NTFF
    profile hook from the boot shim so BASS_TRACE=1 can capture timings."""
    try:
        import antenv.axon_hooks  # noqa: F401
        return
    except ImportError:
        pass
    import types
    import antenv
    hook = None
    try:
        from trn_agent_boot.trn_boot import _ntff_profile_via_ctypes
        if os.path.exists("/opt/axon/libaxon_pjrt.so"):
            hook = _ntff_profile_via_ctypes("/opt/axon/libaxon_pjrt.so")
    except Exception:
        hook = None
    mod = types.ModuleType("antenv.axon_hooks")
    mod.get_axon_ntff_profile_hook = lambda: hook
    mod.set_axon_ntff_profile_hook = lambda h: None
    sys.modules["antenv.axon_hooks"] = mod
    antenv.axon_hooks = mod


def _patch_upload():
    """Don't attempt S3 artifact uploads from the sandbox."""
    import concourse.bass_utils as bu
    bu.upload_artifacts = lambda tmpdir: str(tmpdir)


_ensure_axon_hooks()
_patch_upload()


def _cantor_values(seq_len, depth):
    pos = np.arange(seq_len, dtype=np.float64)
    x = pos / max(1, seq_len - 1)
    x = np.clip(x, 1e-06, 1.0 - 1e-06)
    cantor = np.zeros(seq_len, dtype=np.float64)
    factor = 0.5
    for _ in range(depth):
        x = x * 3.0
        digit = np.floor(x)
        x = x - digit
        cantor += factor * (digit == 2.0)
        factor *= 0.5
    return cantor.astype(np.float32)


def _plan_windows(routes_p):
    """Per 128-query block: list of (key_tile, width) windows.

    Windows are absolute 128-row tiles of the key sequence; the first
    window of a block is always full-width, the last may be a 32-multiple
    prefix. Masks zero out any in-window key not routed, so stale PSUM
    rows in partial windows are harmless (exp of a bounded score * 0).
    """
    lo_all = routes_p.min(axis=1).reshape(NBLK, BLK).min(axis=1)
    hi_all = (routes_p.max(axis=1) + 1).reshape(NBLK, BLK).max(axis=1)
    plans = []
    for b in range(NBLK):
        lo = (int(lo_all[b]) // 128) * 128      # full first tile
        hi32 = int(np.ceil(hi_all[b] / 32.0)) * 32
        a0 = lo // 128
        aL = (hi32 - 1) // 128
        wins = []
        for t in range(a0, aL + 1):
            w = 128 if t < aL else hi32 - 128 * aL
            wins.append((t, w))
        plans.append(tuple(wins))
    return tuple(plans)


def _build_masks(routes_p, plans):
    """Count-masks in device layout [128, 2, nU, BLK] bf16 (head-pair dup,
    hh-major to match the bank-separated score layout).

    mask[p, :, u, q] = multiplicity of key (128*tile_u + p) in the routes
    of query q of the block owning window u.
    """
    parts = []
    for b, wins in enumerate(plans):
        r = routes_p[b * BLK:(b + 1) * BLK]              # [BLK, KNN]
        for (t, w) in wins:
            m = np.zeros((128, BLK), dtype=np.float32)
            rel = r - 128 * t
            sel = (rel >= 0) & (rel < 128)
            qidx = np.broadcast_to(np.arange(BLK)[:, None], rel.shape)
            np.add.at(m, (rel[sel], qidx[sel]), 1.0)
            parts.append(m)
    mk = np.stack(parts, axis=1)                         # [128, nU, BLK]
    mk = np.broadcast_to(mk[:, None, :, :], (128, 2, mk.shape[1], BLK))
    return np.ascontiguousarray(mk).astype(BF16NP)


def _build_program(plans):
    """Emit the SPMD Bass program for the given window plan."""
    nU = sum(len(w) for w in plans)
    nwmax = max(len(w) for w in plans)
    # scores psum tile is [128, 2, 4, BLK]: each hh (PE row group) owns its
    # own PSUM bank — concurrent row-group matmuls must not share a bank.
    assert nwmax <= 4, f"window plan too wide for psum banking: {nwmax}"

    nc = bacc.Bacc("TRN2", target_bir_lowering=False)

    xT_d = nc.dram_tensor("xT", [DIM, S], BF16, kind="ExternalInput")
    wq_d = nc.dram_tensor("wqkvT", [DIM, FQK + FV], BF16, kind="ExternalInput")
    bqk_d = nc.dram_tensor("bqk", [FQK], F32, kind="ExternalInput")
    bv_d = nc.dram_tensor("bv2", [2 * FV], F32, kind="ExternalInput")
    wo_d = nc.dram_tensor("woT", [FV, DIM], BF16, kind="ExternalInput")
    mask_d = nc.dram_tensor("maskT", [128, 2, nU, BLK], BF16, kind="ExternalInput")
    out_d = nc.dram_tensor("out_p", [S, DIM], F16, kind="ExternalOutput")

    with tile.TileContext(nc) as tc:
        with tc.tile_pool(name="const", bufs=1) as cpool, \
             tc.tile_pool(name="work", bufs=1) as wpool, \
             tc.tile_pool(name="epool", bufs=3) as epool:

            # ---- constant loads, interleaved per contraction tile ----
            xT = cpool.tile([128, KT, S], BF16, tag="xT")
            wq = cpool.tile([128, KT, FQK + FV], BF16, tag="wq")
            for kt in range(KT):
                nc.sync.dma_start(
                    wq[:, kt, :],
                    wq_d.rearrange("(t p) f -> p t f", p=128)[:, kt, :])
                nc.sync.dma_start(
                    xT[:, kt, :],
                    xT_d.rearrange("(t p) s -> p t s", p=128)[:, kt, :])
            bqk = cpool.tile([128, FQK // 128], F32, tag="bqk")
            nc.sync.dma_start(bqk[:], bqk_d.rearrange("(t p) -> p t", p=128))
            # v bias pre-duplicated for fused adds over st pairs
            bvb = cpool.tile([128, 2, HPC, HD], F32, tag="bvb")
            nc.sync.dma_start(
                bvb[:],
                bv_d[None, :].to_broadcast((128, 2 * FV)).rearrange(
                    "p (j h d) -> p j h d", j=2, h=HPC))
            wo = cpool.tile([128, 2, DIM], BF16, tag="wo")
            nc.sync.dma_start(wo[:], wo_d.rearrange("(t p) o -> p t o", p=128))
            mk = wpool.tile([128, 2, nU, BLK], BF16, tag="mask")
            nc.sync.dma_start(mk[:], mask_d[:])

            qk_sb = wpool.tile([128, FQK // 128, S], BF16, tag="qk")
            # v layout per head: col 0 = ones (softmax denominator lands on
            # PSUM partition 0), cols 1..63 = zero pad (so attention values
            # land at partition 64 — engine APs spanning 64 partitions must
            # start at partition 0 or 64), cols 64..127 = v.
            VW = 128
            v_sb = wpool.tile([128, S // 128, HPC, VW], BF16, tag="v")
            attnT = wpool.tile([128, 2, S], BF16, tag="attnT")
            nc.gpsimd.memset(v_sb[:, :, :, 0:1], 1.0)
            nc.gpsimd.memset(v_sb[:, :, :, 1:64], 0.0)

            # v tiles required before each block's PV can run
            need_tile = [max(t for t, _ in plans[b]) for b in range(NBLK)]
            for b in range(1, NBLK):
                need_tile[b] = max(need_tile[b], need_tile[b - 1])
            next_sp = 0

            # v projection emitted as dense PE filler: prologue tiles in
            # the phase-A pool, the rest between attention blocks so the
            # HAM activity monitor keeps the PE at full clock.
            def vproj(sp, pool, tag):
                pt = pool.tile([128, 2, 256], F32, tag=tag, name=f"pb{sp}")
                for j in range(2):
                    st = 2 * sp + j
                    for kt in range(KT):
                        nc.tensor.matmul(
                            pt[:, j, :],
                            xT[:, kt, st * 128:(st + 1) * 128],
                            wq[:, kt, FQK:],
                            start=(kt == 0), stop=(kt == KT - 1))
                nc.vector.tensor_add(
                    v_sb[:, 2 * sp:2 * sp + 2, :, 64:64 + HD],
                    pt.rearrange("p j (h d) -> p j h d", h=HPC),
                    bvb[:])

            # ---- phase A: q/k projection -> qk_sb [128, 4, S] ----
            # g-blocked: 8 live psum accumulators, PE weights reused over
            # the 4 512-col column tiles, DMA-paced by the per-kt loads.
            with tc.tile_pool(name="pacc", bufs=8, space="PSUM") as pa:
                for g in range(2):
                    pts = []
                    for f in range(2):
                        for st in range(4):
                            t = pa.tile([128, 512], F32, tag="acc",
                                        name=f"pa{g}{f}{st}")
                            pts.append(t)
                    for kt in range(KT):
                        for f in range(2):
                            ft = 2 * g + f
                            for st in range(4):
                                nc.tensor.matmul(
                                    pts[f * 4 + st][:],
                                    wq[:, kt, ft * 128:(ft + 1) * 128],
                                    xT[:, kt, st * 512:(st + 1) * 512],
                                    start=(kt == 0), stop=(kt == KT - 1))
                    for f in range(2):
                        ft = 2 * g + f
                        for st in range(4):
                            nc.scalar.activation(
                                qk_sb[:, ft, st * 512:(st + 1) * 512],
                                pts[f * 4 + st][:],
                                mybir.ActivationFunctionType.Identity,
                                bias=bqk[:, ft:ft + 1])
                # v-projection prologue: tiles block 0 needs, emitted in
                # this pool so the PE stays dense across the phase boundary
                while next_sp * 2 <= need_tile[0] and next_sp < S // 256:
                    vproj(next_sp, pa, "acc")
                    next_sp += 1

            # ---- attention + v-projection + outproj ----
            # Emission is software-pipelined: per step, attention for block
            # b, normalize for b-1, output projection for b-2. Engines are
            # strict-FIFO queues, so each queue must only see work whose
            # dependencies resolve promptly — interleaving the stages of
            # one block serializes the whole chain per block.
            with tc.tile_pool(name="ps", bufs=2, space="PSUM") as ps, \
                 tc.tile_pool(name="pv", bufs=2, space="PSUM") as pv, \
                 tc.tile_pool(name="po", bufs=2, space="PSUM") as po:
                moffs = np.cumsum([0] + [len(w) for w in plans]).tolist()
                pvts = {}

                def stage1(b):            # QK -> exp -> mask -> PV
                    wins = plans[b]
                    nw = len(wins)
                    qs = slice(b * BLK, (b + 1) * BLK)
                    moff = moffs[b]
                    pvt = pv.tile([128, 4, BLK], F32, tag="pv",
                                  name=f"pv{b}")
                    pvts[b] = pvt
                    for hp in range(2):
                        pst = ps.tile([128, 2, 4, BLK], F32, tag="ps",
                                      name=f"ps{b}{hp}")
                        for hh in range(2):
                            for u, (t, w) in enumerate(wins):
                                nc.tensor.matmul(
                                    pst[0:w, hh, u, :],
                                    qk_sb[64 * hh:64 * hh + 64, 2 + hp,
                                          128 * t:128 * t + w],
                                    qk_sb[64 * hh:64 * hh + 64, hp, qs],
                                    start=True, stop=True)
                        et = epool.tile([128, 2, 4, BLK], BF16, tag="et")
                        nc.scalar.activation(
                            et[:, :, :nw, :], pst[:, :, :nw, :],
                            mybir.ActivationFunctionType.Exp,
                            scale=float(SCALE))
                        emt = epool.tile([128, 2, 4, BLK], BF16, tag="emt")
                        nc.vector.tensor_mul(emt[:, :, :nw, :],
                                             et[:, :, :nw, :],
                                             mk[:, :, moff:moff + nw, :])
                        for hh in range(2):
                            for u, (t, w) in enumerate(wins):
                                nc.tensor.matmul(
                                    pvt[0:VW, 2 * hp + hh, :],
                                    v_sb[0:w, t, 2 * hp + hh, :],
                                    emt[0:w, hh, u, :],
                                    start=(u == 0), stop=(u == nw - 1))

                def stage2(b):            # recip -> broadcast -> scale
                    qs = slice(b * BLK, (b + 1) * BLK)
                    pvt = pvts.pop(b)
                    rr = epool.tile([1, 4, BLK], F32, tag="rr")
                    nc.vector.reciprocal_approx_fast(rr[:], pvt[0:1, :, :])
                    rb = epool.tile([128, 4, BLK], F32, tag="rb")
                    nc.gpsimd.partition_broadcast(rb[:], rr[:])
                    for hh in range(2):
                        nc.vector.tensor_mul(
                            attnT[64 * hh:64 * hh + 64, :, qs],
                            pvt[64:64 + HD, hh::2, :],
                            rb[64 * hh:64 * hh + 64, hh::2, :])

                def stage3(b):            # output projection + store
                    qs = slice(b * BLK, (b + 1) * BLK)
                    ob = epool.tile([128, DIM], F16, tag="ob")
                    for ot in range(2):
                        pot = po.tile([128, 512], F32, tag="po",
                                      name=f"po{b}{ot}")
                        for dt in range(2):
                            nc.tensor.matmul(
                                pot[:],
                                attnT[:, dt, qs],
                                wo[:, dt, ot * 512:(ot + 1) * 512],
                                start=(dt == 0), stop=(dt == 1))
                        if ot == 0:
                            nc.vector.tensor_copy(ob[:, :512], pot[:])
                        else:
                            nc.scalar.copy(ob[:, 512:], pot[:])
                    nc.sync.dma_start(out_d[qs, :], ob[:])

                for b in range(NBLK + 2):
                    if b < NBLK:
                        # keep one block of v-projection lookahead
                        need = need_tile[min(b + 1, NBLK - 1)]
                        if b == NBLK - 1:
                            need = S // 128 - 1
                        while next_sp * 2 <= need and next_sp < S // 256:
                            vproj(next_sp, po, "po")
                            next_sp += 1
                        stage1(b)
                    if 1 <= b < NBLK + 1:
                        stage2(b - 1)
                    if b >= 2:
                        stage3(b - 2)

    nc.finalize()
    return nc


def kernel(x, w_qkv, b_qkv, w_out, b_out, routes):
    global LAST_RESULTS
    x = np.asarray(x, dtype=np.float32)
    w_qkv = np.asarray(w_qkv, dtype=np.float32)
    b_qkv = np.asarray(b_qkv, dtype=np.float32)
    w_out = np.asarray(w_out, dtype=np.float32)
    b_out = np.asarray(b_out, dtype=np.float32)
    routes = np.asarray(routes)

    # --- host: permutation + windows + masks ---
    cantor = _cantor_values(S, DEPTH)
    perm = np.lexsort((np.arange(S), cantor))
    inv_perm = np.empty(S, dtype=np.int64)
    inv_perm[perm] = np.arange(S)
    routes_p = inv_perm[routes.astype(np.int64)[perm]]
    plans = _plan_windows(routes_p)
    maskT = _build_masks(routes_p, plans)

    key = plans
    if key not in _PROGRAM_CACHE:
        _PROGRAM_CACHE[key] = _build_program(plans)
    nc = _PROGRAM_CACHE[key]

    # --- host: per-core inputs ---
    x_p = x[:, perm, :]                                   # [B, S, DIM]
    in_maps = []
    for c in range(N_CORES):
        b = c // (N_CORES // B)
        hg = c % (N_CORES // B)
        heads = range(hg * HPC, (hg + 1) * HPC)
        # w rows: q heads, k heads, v heads
        rows = ([h * HD + i for h in heads for i in range(HD)]
                + [DIM + h * HD + i for h in heads for i in range(HD)]
                + [2 * DIM + h * HD + i for h in heads for i in range(HD)])
        rows = np.asarray(rows)
        wq_c = np.ascontiguousarray(w_qkv[rows].T).astype(BF16NP)   # [1024, 768]
        bqk_c = np.ascontiguousarray(b_qkv[rows[:FQK]]).astype(np.float32)
        bv_c = np.ascontiguousarray(b_qkv[rows[FQK:]]).astype(np.float32)
        bv2_c = np.concatenate([bv_c, bv_c])
        wo_c = np.ascontiguousarray(
            w_out[:, hg * FV:(hg + 1) * FV].T).astype(BF16NP)
        in_maps.append({
            "xT": np.ascontiguousarray(x_p[b].T).astype(BF16NP),
            "wqkvT": wq_c,
            "bqk": bqk_c,
            "bv2": bv2_c,
            "woT": wo_c,
            "maskT": maskT,
        })

    try:
        res = run_bass_kernel_spmd(nc, in_maps, core_ids=list(range(N_CORES)))
    except Exception:
        if os.environ.get("BASS_TRACE"):
            # tracing infra failure — retry without profiling
            os.environ["BASS_NEVER_TRACE"] = "1"
            res = run_bass_kernel_spmd(nc, in_maps, core_ids=list(range(N_CORES)))
        else:
            raise
    LAST_RESULTS = res

    out = np.zeros((B, S, DIM), dtype=np.float32)
    for c in range(N_CORES):
        out[c // (N_CORES // B)] += res.results[c]["out_p"].astype(np.float32)
    out += b_out[None, None, :]
    out = out[:, inv_perm, :]    # un-permute rows
    return out
